# revision 9
# baseline (speedup 1.0000x reference)
"""Trainium2 Bass kernel for nn_DistillingLayer: per-channel shared-weight
Conv1d(k=3, stride=2, pad=1) + ELU + MaxPool1d(k=3, stride=2, pad=1) over
x:(16, 4096, 512) f32 -> out:(16, 1024, 512) f32.

Strategy (v2 — fp16 on-chip, DMA-roofline focused)
--------------------------------------------------
- Data-parallel over batch: 8 cores x 2 batches each. No communication.
- The kernel is HBM-bound (must read 16.8 MB + write out per core at
  ~358 GB/s/NC). v1 was jointly Vector- and DMA-limited (VectorE busy
  71 us, DMA 68 us, e2e 103 us). v2 computes in fp16 so every DVE op
  runs in the 2x_1P perf mode, halving VectorE time and leaving the
  input DMA stream as the only bottleneck.
- Layout: L lives in the SBUF free dimension. One tile per batch:
  each partition owns S=32 consecutive L-rows (x D=512 channels) plus a
  3-row halo loaded with overlap from HBM (9.4% overhead vs 18.75% for
  v1's S=16), so conv+pool stay per-partition local.
- The input is zero-padded by 3 L-rows on the host: uniform
  full-128-partition DMAs and free conv left-padding.
- Input DMAs run on the gpsimd (SWDGE) queue and cast f32->fp16 in the
  SDMA datapath. Each tile is split into 4-5 row-chunks so compute
  starts after the first chunk lands and the kernel tail is short.
- ELU is monotonic, so maxpool commutes: pool pre-activation conv
  outputs, ELU once on the pooled rows. Conv bias is folded into the
  first tap (ScalarE Copy activation applies scale+bias); taps 2/3 are
  VectorE scalar_tensor_tensor accumulates; pool is two VectorE max
  passes; ELU(v) = max(exp(min(v,0))-1, v) via two ScalarE activations
  + one fused VectorE scalar_tensor_tensor.
- Outputs are stored as fp16 via the sync (HWDGE) queue — it runs in
  parallel with the SWDGE input queue, so stores never block loads —
  and upcast to f32 on the host (absmax-scaled error ~1e-3, gate 2e-2).
- Weights/bias are baked as immediates; the compiled module is cached
  per (w, b) value.

Toolchain workaround (see inline comment): a BIR post-pass splits
multi-wait instructions — this walrus build allows one sync wait per
instruction.
"""

import json as _json
import os
import sys

import numpy as np

for _p in ("/opt/trn_rl_repo", "/root/.axon_site/_ro/trn_rl_repo"):
    if os.path.isdir(_p) and _p not in sys.path:
        sys.path.append(_p)

import concourse.bass as bass
import concourse.bass2jax as bass2jax
import concourse.bass_utils as bass_utils
import concourse.mybir as mybir
from concourse.bass_utils import run_bass_kernel_spmd
from concourse.tile import TileContext

# ---------------------------------------------------------------------------
# REQUIRED workaround: this container's walrus build rejects instructions
# carrying more than one sync wait ("Too many sync wait commands" in
# setupSyncWait). Tile's scheduler freely attaches several waits to one
# instruction, so post-process the BIR JSON before compile: hoist all but the
# last wait onto same-engine NoOps inserted just before the instruction
# (per-engine program order makes sequential waits equivalent to a
# multi-wait).
# ---------------------------------------------------------------------------

_orig_compile_bir_kernel = bass_utils.compile_bir_kernel


def _split_multi_waits(bir_json: bytes) -> bytes:
    j = _json.loads(bir_json)
    ctr = 0
    changed = False
    for fn in j["functions"]:
        for bb in fn["blocks"]:
            out = []
            for ins in bb["instructions"]:
                si = ins.get("sync_info")
                waits = (si.get("on_wait") or []) if si else []
                if len(waits) > 1:
                    changed = True
                    for w in waits[:-1]:
                        ctr += 1
                        out.append(
                            {
                                "debug": ins.get("debug", 0),
                                "engine": ins["engine"],
                                "ins": [],
                                "outs": [],
                                "name": f"waitsplit-{ctr}",
                                "opcode": "NoOp",
                                "text_hint": "waitsplit",
                                "sync_info": {"on_update": [], "on_wait": [w]},
                            }
                        )
                    si["on_wait"] = [waits[-1]]
                out.append(ins)
            bb["instructions"] = out
    if not changed:
        return bir_json
    return _json.dumps(j).encode()


def _patched_compile_bir_kernel(bir_json, tmpdir, neff_name="file.neff"):
    return _orig_compile_bir_kernel(_split_multi_waits(bir_json), tmpdir, neff_name)


bass_utils.compile_bir_kernel = _patched_compile_bir_kernel
bass2jax.compile_bir_kernel = _patched_compile_bir_kernel

# The first TileContext exit barrier's per-engine drains are redundant (the
# tail waits already cover all completions); use the cheap sequencer-level
# variant there. The SECOND barrier stays full — its drains restore
# engine/queue state so the loaded NEFF can re-execute.
try:
    from concourse.vector_clock import ScopedClock as _ScopedClock

    def _tail_drain_and_barrier(self, tick_clock, wait_clock):
        drain_inst = self.nc.sync.drain()
        wait_clock.add_sem_waits(
            drain_inst.ins, _ScopedClock({None: tick_clock.global_clock})
        )
        self.nc.all_engine_barrier(sem_only=True)
        assert self.sems is not None
        popped = self.nc._tile_sem_poison_stack.pop()
        assert popped is self._sem_poison
        self.nc.clear_and_free_semaphores(list(self.sems.allocated().values()))
        self.nc.all_engine_barrier()

    TileContext._drain_and_barrier = _tail_drain_and_barrier
except Exception:
    pass

# ---------------------------------------------------------------------------

N_CORES = 8
B, L, D = 16, 4096, 512
BPC = B // N_CORES  # batches per core
LC = L // 2         # conv output length
LP = LC // 2        # pool output length
S = 32              # input L-rows owned per partition (128 * 32 = 4096)
Q = S // 2 + 1      # conv rows per partition (incl. 1 left-halo conv row)
JT = S // 4         # pool-output rows per partition

F32 = mybir.dt.float32
F16 = mybir.dt.float16
ALU = mybir.AluOpType
AF = mybir.ActivationFunctionType

_cache: dict = {}

# Exposed for test harnesses: the BassKernelResults of the last run.
LAST_RESULT = None


def _build(w0: float, w1: float, w2: float, bias: float) -> bass.Bass:
    nc = bass.Bass()
    # x is host-padded with 3 zero rows at the front of L: padded row r
    # holds true row r-3 (see module docstring).
    x = nc.dram_tensor("x", [BPC, L + 3, D], F32, kind="ExternalInput")
    y = nc.dram_tensor("y", [BPC, LP, D], F16, kind="ExternalOutput")

    xrow = D              # elements per L-row
    xbat = (L + 3) * D    # elements per (padded) input batch
    ybat = LP * D

    with TileContext(nc) as tc:
        with (
            tc.tile_pool(name="xp", bufs=2) as xp,
            tc.tile_pool(name="yp", bufs=2) as yp,
            tc.tile_pool(name="up", bufs=3) as up,
            tc.tile_pool(name="tp", bufs=3) as tp,
            tc.tile_pool(name="pp", bufs=2) as pp,
            tc.tile_pool(name="rp", bufs=2) as rp,
        ):
            for b in range(BPC):
                last = b == BPC - 1
                # Input row-chunks, conv q-segments and pool j-segments are
                # aligned so each conv segment only needs already-landed
                # chunks (conv q taps local rows [2qa, 2qb+1)) and each pool
                # segment only needs finished conv rows (q in [2ja, 2jb+1)).
                # The last tile gets a finer tail so the final
                # load->conv->pool->store chain after the last chunk is short.
                if last:
                    chunks = [(0, 5), (5, 13), (13, 21), (21, 29), (29, 33), (33, 35)]
                    poolsegs = [(0, 4), (4, 6), (6, 8)]
                else:
                    chunks = [(0, 5), (5, 13), (13, 21), (21, 29), (29, 35)]
                    poolsegs = [(0, 4), (4, 8)]
                convsegs = [(0, 2), (2, 6), (6, 10), (10, 14), (14, 16), (16, 17)]

                # Partition p holds padded rows [32p, 32p+35) = true rows
                # [32p-3, 32p+32): 3 halo rows, then its own 32 rows.
                X = xp.tile([128, (S + 3) * D], F16)
                Xv = X[:, :].rearrange("p (r d) -> p r d", d=D)
                Y = yp.tile([128, Q * D], F16)
                y3 = Y[:, :].rearrange("p (q d) -> p q d", d=D)
                P = pp.tile([128, JT * D], F16)
                p3 = P[:, :].rearrange("p (j d) -> p j d", d=D)
                R = rp.tile([128, JT * D], F16)

                for r0, r1 in chunks:
                    nc.gpsimd.dma_start(
                        out=X[:, r0 * D : r1 * D],
                        in_=bass.AP(
                            x,
                            b * xbat + r0 * xrow,
                            [[S * xrow, 128], [1, (r1 - r0) * xrow]],
                        ),
                    )

                # conv, shifted by +1 (the host subtracts 1 from the final
                # output): partition p's conv row q (local) is
                # c[16p - 1 + q] = w0*x[2q] + w1*x[2q+1] + w2*x[2q+2] + bias+1
                # (x indices local to the partition's 35-row strip).
                # scalar_tensor_tensor only has a 1x DVE uop, so the taps are
                # built from 4x tensor_scalar mults (even segment sizes only;
                # odd row counts drop TS to 2x) + 2x tensor_tensor adds.
                # Balance: ScalarE takes the w0 (with bias) and w2 mults, DVE
                # the w1 mult and both adds. ys += U runs first: it only
                # needs the w0/w1 products, not ScalarE's later w2 pass.
                # The global left pool pad (q=0 of partition 0, conv over the
                # 3 host pad rows) is forced very negative by sign-based pad
                # values chosen on the host, so no -inf memset is needed.
                for qa, qb in convsegs:
                    nq = qb - qa
                    ya = Xv[:, 2 * qa : 2 * qb - 1 : 2, :]
                    yb = Xv[:, 2 * qa + 1 : 2 * qb : 2, :]
                    yc = Xv[:, 2 * qa + 2 : 2 * qb + 1 : 2, :]
                    ys = y3[:, qa:qb, :]
                    U = up.tile([128, nq * D], F16)
                    U3 = U[:, :].rearrange("p (q d) -> p q d", d=D)
                    T = tp.tile([128, nq * D], F16)
                    T3 = T[:, :].rearrange("p (q d) -> p q d", d=D)
                    nc.scalar.activation(ys, ya, AF.Copy, bias=bias + 1.0, scale=w0)
                    nc.vector.tensor_scalar(U3, yb, w1, None, op0=ALU.mult)
                    nc.scalar.activation(T3, yc, AF.Copy, bias=0.0, scale=w2)
                    nc.vector.tensor_tensor(ys, ys, U3, op=ALU.add)
                    nc.vector.tensor_tensor(ys, ys, T3, op=ALU.add)

                # maxpool (pre-activation; ELU is monotonic), all shifted +1:
                # P[8p + j] = max(y3[2j], y3[2j+1], y3[2j+2]) = v + 1; then
                # out+1 = max(exp(min(v,0)), v+1), via one 4x dual-op
                # tensor_scalar (m = min(P-1, 0)), one ScalarE Exp, one 2x
                # tensor_tensor max. Stores go out fp16 on the sync (HWDGE)
                # queue, parallel to the SWDGE input queue.
                for ja, jb in poolsegs:
                    ps = p3[:, ja:jb, :]
                    pf = P[:, ja * D : jb * D]
                    rs = R[:, ja * D : jb * D]
                    nc.vector.tensor_tensor(
                        ps,
                        y3[:, 2 * ja : 2 * jb - 1 : 2, :],
                        y3[:, 2 * ja + 1 : 2 * jb : 2, :],
                        op=ALU.max,
                    )
                    nc.vector.tensor_tensor(
                        ps, ps, y3[:, 2 * ja + 2 : 2 * jb + 1 : 2, :], op=ALU.max
                    )
                    nc.vector.tensor_scalar(
                        rs, pf, -1.0, 0.0, op0=ALU.add, op1=ALU.min
                    )
                    nc.scalar.activation(rs, rs, AF.Exp)
                    nc.vector.tensor_tensor(rs, rs, pf, op=ALU.max)
                    nc.sync.dma_start(
                        out=bass.AP(
                            y,
                            b * ybat + ja * xrow,
                            [[JT * xrow, 128], [1, (jb - ja) * xrow]],
                        ),
                        in_=rs,
                    )
    return nc


def kernel(x: np.ndarray, w: np.ndarray, b: np.ndarray) -> np.ndarray:
    global LAST_RESULT
    w = np.asarray(w, dtype=np.float32)
    bb = np.asarray(b, dtype=np.float32)
    key = (float(w[0]), float(w[1]), float(w[2]), float(bb[0]))
    if key not in _cache:
        _cache[key] = _build(*key)
    nc = _cache[key]

    x = np.asarray(x, dtype=np.float32)
    assert x.shape == (B, L, D), x.shape
    xpad = np.zeros((B, L + 3, D), dtype=np.float32)
    xpad[:, 3:, :] = x
    # Pad rows 0/1 are chosen so the left pool pad c[-1] = w0*p0 + w1*p1 +
    # bias + 1 is hugely negative (it must lose every max against real conv
    # values; the reference excludes the pool pad via -inf). Row 2 stays 0:
    # it is the conv's own zero left-pad, used by c[0].
    w0, w1 = float(w[0]), float(w[1])
    C = min(60000.0, 40000.0 / max(abs(w0) + abs(w1), 1e-3))
    xpad[:, 0, :] = -np.sign(w0) * C if w0 != 0.0 else 0.0
    xpad[:, 1, :] = -np.sign(w1) * C if w1 != 0.0 else 0.0
    in_maps = [
        {"x": np.ascontiguousarray(xpad[c * BPC : (c + 1) * BPC])}
        for c in range(N_CORES)
    ]
    res = run_bass_kernel_spmd(nc, in_maps, core_ids=list(range(N_CORES)))
    LAST_RESULT = res
    out = np.concatenate([r["y"] for r in res.results], axis=0)
    # device computes out+1 in fp16 (see _build); undo the shift here
    return out.astype(np.float32) - 1.0


# revision 16
# speedup vs baseline: 1.0503x; 1.0503x over previous
"""Trainium2 Bass kernel for nn_DistillingLayer: per-channel shared-weight
Conv1d(k=3, stride=2, pad=1) + ELU + MaxPool1d(k=3, stride=2, pad=1) over
x:(16, 4096, 512) f32 -> out:(16, 1024, 512) f32.

Strategy (v2 — fp16 on-chip, DMA-roofline focused)
--------------------------------------------------
- Data-parallel over batch: 8 cores x 2 batches each. No communication.
- The kernel is HBM-bound (must read 16.8 MB + write out per core at
  ~358 GB/s/NC). v1 was jointly Vector- and DMA-limited (VectorE busy
  71 us, DMA 68 us, e2e 103 us). v2 computes in fp16 so every DVE op
  runs in the 2x_1P perf mode, halving VectorE time and leaving the
  input DMA stream as the only bottleneck.
- Layout: L lives in the SBUF free dimension. One tile per batch:
  each partition owns S=32 consecutive L-rows (x D=512 channels) plus a
  3-row halo loaded with overlap from HBM (9.4% overhead vs 18.75% for
  v1's S=16), so conv+pool stay per-partition local.
- The input is zero-padded by 3 L-rows on the host: uniform
  full-128-partition DMAs and free conv left-padding.
- Input DMAs run on the gpsimd (SWDGE) queue and cast f32->fp16 in the
  SDMA datapath. Each tile is split into 4-5 row-chunks so compute
  starts after the first chunk lands and the kernel tail is short.
- ELU is monotonic, so maxpool commutes: pool pre-activation conv
  outputs, ELU once on the pooled rows. Conv bias is folded into the
  first tap (ScalarE Copy activation applies scale+bias); taps 2/3 are
  VectorE scalar_tensor_tensor accumulates; pool is two VectorE max
  passes; ELU(v) = max(exp(min(v,0))-1, v) via two ScalarE activations
  + one fused VectorE scalar_tensor_tensor.
- Outputs are stored as fp16 via the sync (HWDGE) queue — it runs in
  parallel with the SWDGE input queue, so stores never block loads —
  and upcast to f32 on the host (absmax-scaled error ~1e-3, gate 2e-2).
- Weights/bias are baked as immediates; the compiled module is cached
  per (w, b) value.

Toolchain workaround (see inline comment): a BIR post-pass splits
multi-wait instructions — this walrus build allows one sync wait per
instruction.
"""

import json as _json
import os
import sys

import numpy as np

for _p in ("/opt/trn_rl_repo", "/root/.axon_site/_ro/trn_rl_repo"):
    if os.path.isdir(_p) and _p not in sys.path:
        sys.path.append(_p)

import concourse.bass as bass
import concourse.bass2jax as bass2jax
import concourse.bass_utils as bass_utils
import concourse.mybir as mybir
from concourse.bass_utils import run_bass_kernel_spmd
from concourse.tile import TileContext

# ---------------------------------------------------------------------------
# REQUIRED workaround: this container's walrus build rejects instructions
# carrying more than one sync wait ("Too many sync wait commands" in
# setupSyncWait). Tile's scheduler freely attaches several waits to one
# instruction, so post-process the BIR JSON before compile: hoist all but the
# last wait onto same-engine NoOps inserted just before the instruction
# (per-engine program order makes sequential waits equivalent to a
# multi-wait).
# ---------------------------------------------------------------------------

_orig_compile_bir_kernel = bass_utils.compile_bir_kernel


def _split_multi_waits(bir_json: bytes) -> bytes:
    j = _json.loads(bir_json)
    ctr = 0
    changed = False
    for fn in j["functions"]:
        for bb in fn["blocks"]:
            out = []
            for ins in bb["instructions"]:
                si = ins.get("sync_info")
                waits = (si.get("on_wait") or []) if si else []
                if len(waits) > 1:
                    changed = True
                    for w in waits[:-1]:
                        ctr += 1
                        out.append(
                            {
                                "debug": ins.get("debug", 0),
                                "engine": ins["engine"],
                                "ins": [],
                                "outs": [],
                                "name": f"waitsplit-{ctr}",
                                "opcode": "NoOp",
                                "text_hint": "waitsplit",
                                "sync_info": {"on_update": [], "on_wait": [w]},
                            }
                        )
                    si["on_wait"] = [waits[-1]]
                out.append(ins)
            bb["instructions"] = out
    if not changed:
        return bir_json
    return _json.dumps(j).encode()


def _patched_compile_bir_kernel(bir_json, tmpdir, neff_name="file.neff"):
    return _orig_compile_bir_kernel(_split_multi_waits(bir_json), tmpdir, neff_name)


bass_utils.compile_bir_kernel = _patched_compile_bir_kernel
bass2jax.compile_bir_kernel = _patched_compile_bir_kernel

# The first TileContext exit barrier's per-engine drains are redundant (the
# tail waits already cover all completions); use the cheap sequencer-level
# variant there. The SECOND barrier stays full — its drains restore
# engine/queue state so the loaded NEFF can re-execute.
try:
    from concourse.vector_clock import ScopedClock as _ScopedClock

    def _tail_drain_and_barrier(self, tick_clock, wait_clock):
        drain_inst = self.nc.sync.drain()
        wait_clock.add_sem_waits(
            drain_inst.ins, _ScopedClock({None: tick_clock.global_clock})
        )
        self.nc.all_engine_barrier(sem_only=True)
        assert self.sems is not None
        popped = self.nc._tile_sem_poison_stack.pop()
        assert popped is self._sem_poison
        self.nc.clear_and_free_semaphores(list(self.sems.allocated().values()))
        self.nc.all_engine_barrier()

    TileContext._drain_and_barrier = _tail_drain_and_barrier
except Exception:
    pass

# ---------------------------------------------------------------------------

N_CORES = 8
B, L, D = 16, 4096, 512
BPC = B // N_CORES  # batches per core
LC = L // 2         # conv output length
LP = LC // 2        # pool output length
S = 32              # input L-rows owned per partition (128 * 32 = 4096)
Q = S // 2 + 1      # conv rows per partition (incl. 1 left-halo conv row)
JT = S // 4         # pool-output rows per partition

F32 = mybir.dt.float32
F16 = mybir.dt.float16
ALU = mybir.AluOpType
AF = mybir.ActivationFunctionType

_cache: dict = {}

# Exposed for test harnesses: the BassKernelResults of the last run.
LAST_RESULT = None


def _build(w0: float, w1: float, w2: float, bias: float) -> bass.Bass:
    nc = bass.Bass()
    # x is host-padded with 3 rows at the front of L: padded row r holds
    # true row r-3 (see module docstring).
    x = nc.dram_tensor("x", [BPC, L + 3, D], F32, kind="ExternalInput")
    # wd holds the three 128x128 diagonal matrices w_k * I (fp16), used as
    # stationary operands so TensorE computes the conv taps as accumulating
    # "matmuls" (diag(w) @ X == w * X elementwise, partition-preserving).
    wd = nc.dram_tensor("wd", [128, 3 * 128], F16, kind="ExternalInput")
    y = nc.dram_tensor("y", [BPC, LP, D], F16, kind="ExternalOutput")

    xrow = D              # elements per L-row
    xbat = (L + 3) * D    # elements per (padded) input batch
    ybat = LP * D

    with TileContext(nc) as tc:
        with (
            tc.tile_pool(name="xp", bufs=2) as xp,
            tc.tile_pool(name="yp", bufs=2) as yp,
            tc.tile_pool(name="wp", bufs=1) as wp,
            tc.tile_pool(name="cp", bufs=2, space="PSUM") as cp,
            tc.tile_pool(name="pp", bufs=2) as pp,
            tc.tile_pool(name="rp", bufs=2) as rp,
        ):
            # The three stationary diag(w_k) matrices, loaded once up front
            # on the sync (HWDGE) queue so the SWDGE input stream is not
            # delayed.
            WD = wp.tile([128, 3 * 128], F16)
            nc.sync.dma_start(
                out=WD[:, :],
                in_=bass.AP(wd, 0, [[3 * 128, 128], [1, 3 * 128]]),
            )
            for b in range(BPC):
                last = b == BPC - 1
                # Input row-chunks, conv q-segments and pool j-segments are
                # aligned so each conv segment only needs already-landed
                # chunks (conv q taps local rows [2qa, 2qb+1)) and each pool
                # segment only needs finished conv rows (q in [2ja, 2jb+1)).
                # The last tile gets a finer tail so the final
                # load->conv->pool->store chain after the last chunk is short.
                if last:
                    chunks = [(0, 5), (5, 13), (13, 21), (21, 29), (29, 33), (33, 35)]
                    poolsegs = [(0, 4), (4, 6), (6, 8)]
                else:
                    chunks = [(0, 5), (5, 13), (13, 21), (21, 29), (29, 35)]
                    poolsegs = [(0, 4), (4, 8)]
                convsegs = [(0, 4), (4, 8), (8, 12), (12, 16), (16, 17)]

                # Partition p holds padded rows [32p, 32p+35) = true rows
                # [32p-3, 32p+32): 3 halo rows, then its own 32 rows.
                X = xp.tile([128, (S + 3) * D], F16)
                Xv = X[:, :].rearrange("p (r d) -> p r d", d=D)
                Y = yp.tile([128, Q * D], F16)
                y3 = Y[:, :].rearrange("p (q d) -> p q d", d=D)
                P = pp.tile([128, JT * D], F16)
                p3 = P[:, :].rearrange("p (j d) -> p j d", d=D)
                R = rp.tile([128, JT * D], F16)

                for r0, r1 in chunks:
                    nc.gpsimd.dma_start(
                        out=X[:, r0 * D : r1 * D],
                        in_=bass.AP(
                            x,
                            b * xbat + r0 * xrow,
                            [[S * xrow, 128], [1, (r1 - r0) * xrow]],
                        ),
                    )

                # conv, shifted by +1 (the host subtracts 1 from the final
                # output): partition p's conv row q (local) is
                # c[16p - 1 + q] = w0*x[2q] + w1*x[2q+1] + w2*x[2q+2] + bias+1
                # (x indices local to the partition's 35-row strip).
                # The taps run on the otherwise-idle TensorE: diag(w_k) as
                # the stationary makes a matmul a partition-preserving
                # elementwise scale, and the three taps accumulate in a PSUM
                # bank (fp32). Matmuls are grouped by tap so the stationary
                # is swapped 3x per wave, not per row. ScalarE then evicts
                # PSUM -> fp16 SBUF, folding in bias+1 via the activation
                # bias. The global left pool pad (q=0 of partition 0, conv
                # over the 3 host pad rows) is forced very negative by
                # sign-based pad values chosen on the host, so no -inf
                # memset is needed.
                for qa, qb in convsegs:
                    nq = qb - qa
                    C4 = cp.tile([128, nq * 512], F32)
                    for k in range(3):
                        Wk = WD[:, k * 128 : (k + 1) * 128]
                        for q in range(qa, qb):
                            nc.tensor.matmul(
                                C4[:, (q - qa) * 512 : (q - qa + 1) * 512],
                                Wk,
                                Xv[:, 2 * q + k, :],
                                start=(k == 0),
                                stop=(k == 2),
                            )
                    nc.scalar.activation(
                        Y[:, qa * D : qb * D], C4[:, :], AF.Copy, bias=bias + 1.0
                    )

                # maxpool (pre-activation; ELU is monotonic), all shifted +1:
                # P[8p + j] = max(y3[2j], y3[2j+1], y3[2j+2]) = v + 1; then
                # out+1 = max(exp(min(v,0)), v+1), via one 4x dual-op
                # tensor_scalar (m = min(P-1, 0)), one ScalarE Exp, one 2x
                # tensor_tensor max. Stores go out fp16 on the sync (HWDGE)
                # queue, parallel to the SWDGE input queue.
                for ja, jb in poolsegs:
                    ps = p3[:, ja:jb, :]
                    pf = P[:, ja * D : jb * D]
                    rs = R[:, ja * D : jb * D]
                    nc.vector.tensor_tensor(
                        ps,
                        y3[:, 2 * ja : 2 * jb - 1 : 2, :],
                        y3[:, 2 * ja + 1 : 2 * jb : 2, :],
                        op=ALU.max,
                    )
                    nc.vector.tensor_tensor(
                        ps, ps, y3[:, 2 * ja + 2 : 2 * jb + 1 : 2, :], op=ALU.max
                    )
                    nc.vector.tensor_scalar(
                        rs, pf, -1.0, 0.0, op0=ALU.add, op1=ALU.min
                    )
                    nc.scalar.activation(rs, rs, AF.Exp)
                    nc.vector.tensor_tensor(rs, rs, pf, op=ALU.max)
                    nc.sync.dma_start(
                        out=bass.AP(
                            y,
                            b * ybat + ja * xrow,
                            [[JT * xrow, 128], [1, (jb - ja) * xrow]],
                        ),
                        in_=rs,
                    )
    return nc


def kernel(x: np.ndarray, w: np.ndarray, b: np.ndarray) -> np.ndarray:
    global LAST_RESULT
    w = np.asarray(w, dtype=np.float32)
    bb = np.asarray(b, dtype=np.float32)
    key = (float(w[0]), float(w[1]), float(w[2]), float(bb[0]))
    if key not in _cache:
        _cache[key] = _build(*key)
    nc = _cache[key]

    x = np.asarray(x, dtype=np.float32)
    assert x.shape == (B, L, D), x.shape
    xpad = np.zeros((B, L + 3, D), dtype=np.float32)
    xpad[:, 3:, :] = x
    # Pad rows 0/1 are chosen so the left pool pad c[-1] = w0*p0 + w1*p1 +
    # bias + 1 is hugely negative (it must lose every max against real conv
    # values; the reference excludes the pool pad via -inf). Row 2 stays 0:
    # it is the conv's own zero left-pad, used by c[0].
    w0, w1 = float(w[0]), float(w[1])
    C = min(60000.0, 40000.0 / max(abs(w0) + abs(w1), 1e-3))
    xpad[:, 0, :] = -np.sign(w0) * C if w0 != 0.0 else 0.0
    xpad[:, 1, :] = -np.sign(w1) * C if w1 != 0.0 else 0.0
    wdiag = np.concatenate(
        [np.eye(128, dtype=np.float16) * np.float16(w[k]) for k in range(3)],
        axis=1,
    )
    in_maps = [
        {
            "x": np.ascontiguousarray(xpad[c * BPC : (c + 1) * BPC]),
            "wd": wdiag,
        }
        for c in range(N_CORES)
    ]
    res = run_bass_kernel_spmd(nc, in_maps, core_ids=list(range(N_CORES)))
    LAST_RESULT = res
    out = np.concatenate([r["y"] for r in res.results], axis=0)
    # device computes out+1 in fp16 (see _build); undo the shift here
    return out.astype(np.float32) - 1.0


# revision 17
# speedup vs baseline: 1.0586x; 1.0079x over previous
"""Trainium2 Bass kernel for nn_DistillingLayer: per-channel shared-weight
Conv1d(k=3, stride=2, pad=1) + ELU + MaxPool1d(k=3, stride=2, pad=1) over
x:(16, 4096, 512) f32 -> out:(16, 1024, 512) f32.

Strategy (v2 — fp16 on-chip, DMA-roofline focused)
--------------------------------------------------
- Data-parallel over batch: 8 cores x 2 batches each. No communication.
- The kernel is HBM-bound (must read 16.8 MB + write out per core at
  ~358 GB/s/NC). v1 was jointly Vector- and DMA-limited (VectorE busy
  71 us, DMA 68 us, e2e 103 us). v2 computes in fp16 so every DVE op
  runs in the 2x_1P perf mode, halving VectorE time and leaving the
  input DMA stream as the only bottleneck.
- Layout: L lives in the SBUF free dimension. One tile per batch:
  each partition owns S=32 consecutive L-rows (x D=512 channels) plus a
  3-row halo loaded with overlap from HBM (9.4% overhead vs 18.75% for
  v1's S=16), so conv+pool stay per-partition local.
- The input is zero-padded by 3 L-rows on the host: uniform
  full-128-partition DMAs and free conv left-padding.
- Input DMAs run on the gpsimd (SWDGE) queue and cast f32->fp16 in the
  SDMA datapath. Each tile is split into 4-5 row-chunks so compute
  starts after the first chunk lands and the kernel tail is short.
- ELU is monotonic, so maxpool commutes: pool pre-activation conv
  outputs, ELU once on the pooled rows. Conv bias is folded into the
  first tap (ScalarE Copy activation applies scale+bias); taps 2/3 are
  VectorE scalar_tensor_tensor accumulates; pool is two VectorE max
  passes; ELU(v) = max(exp(min(v,0))-1, v) via two ScalarE activations
  + one fused VectorE scalar_tensor_tensor.
- Outputs are stored as fp16 via the sync (HWDGE) queue — it runs in
  parallel with the SWDGE input queue, so stores never block loads —
  and upcast to f32 on the host (absmax-scaled error ~1e-3, gate 2e-2).
- Weights/bias are baked as immediates; the compiled module is cached
  per (w, b) value.

Toolchain workaround (see inline comment): a BIR post-pass splits
multi-wait instructions — this walrus build allows one sync wait per
instruction.
"""

import json as _json
import os
import sys

import numpy as np

for _p in ("/opt/trn_rl_repo", "/root/.axon_site/_ro/trn_rl_repo"):
    if os.path.isdir(_p) and _p not in sys.path:
        sys.path.append(_p)

import concourse.bass as bass
import concourse.bass2jax as bass2jax
import concourse.bass_utils as bass_utils
import concourse.mybir as mybir
from concourse.bass_utils import run_bass_kernel_spmd
from concourse.tile import TileContext

# ---------------------------------------------------------------------------
# REQUIRED workaround: this container's walrus build rejects instructions
# carrying more than one sync wait ("Too many sync wait commands" in
# setupSyncWait). Tile's scheduler freely attaches several waits to one
# instruction, so post-process the BIR JSON before compile: hoist all but the
# last wait onto same-engine NoOps inserted just before the instruction
# (per-engine program order makes sequential waits equivalent to a
# multi-wait).
# ---------------------------------------------------------------------------

_orig_compile_bir_kernel = bass_utils.compile_bir_kernel


def _split_multi_waits(bir_json: bytes) -> bytes:
    j = _json.loads(bir_json)
    ctr = 0
    changed = False
    for fn in j["functions"]:
        for bb in fn["blocks"]:
            out = []
            for ins in bb["instructions"]:
                si = ins.get("sync_info")
                waits = (si.get("on_wait") or []) if si else []
                if len(waits) > 1:
                    changed = True
                    for w in waits[:-1]:
                        ctr += 1
                        out.append(
                            {
                                "debug": ins.get("debug", 0),
                                "engine": ins["engine"],
                                "ins": [],
                                "outs": [],
                                "name": f"waitsplit-{ctr}",
                                "opcode": "NoOp",
                                "text_hint": "waitsplit",
                                "sync_info": {"on_update": [], "on_wait": [w]},
                            }
                        )
                    si["on_wait"] = [waits[-1]]
                out.append(ins)
            bb["instructions"] = out
    if not changed:
        return bir_json
    return _json.dumps(j).encode()


def _patched_compile_bir_kernel(bir_json, tmpdir, neff_name="file.neff"):
    return _orig_compile_bir_kernel(_split_multi_waits(bir_json), tmpdir, neff_name)


bass_utils.compile_bir_kernel = _patched_compile_bir_kernel
bass2jax.compile_bir_kernel = _patched_compile_bir_kernel

# The first TileContext exit barrier's per-engine drains are redundant (the
# tail waits already cover all completions); use the cheap sequencer-level
# variant there. The SECOND barrier stays full — its drains restore
# engine/queue state so the loaded NEFF can re-execute.
try:
    from concourse.vector_clock import ScopedClock as _ScopedClock

    def _tail_drain_and_barrier(self, tick_clock, wait_clock):
        drain_inst = self.nc.sync.drain()
        wait_clock.add_sem_waits(
            drain_inst.ins, _ScopedClock({None: tick_clock.global_clock})
        )
        self.nc.all_engine_barrier(sem_only=True)
        assert self.sems is not None
        popped = self.nc._tile_sem_poison_stack.pop()
        assert popped is self._sem_poison
        self.nc.clear_and_free_semaphores(list(self.sems.allocated().values()))
        self.nc.all_engine_barrier()

    TileContext._drain_and_barrier = _tail_drain_and_barrier
except Exception:
    pass

# ---------------------------------------------------------------------------

N_CORES = 8
B, L, D = 16, 4096, 512
BPC = B // N_CORES  # batches per core
LC = L // 2         # conv output length
LP = LC // 2        # pool output length
S = 32              # input L-rows owned per partition (128 * 32 = 4096)
Q = S // 2 + 1      # conv rows per partition (incl. 1 left-halo conv row)
JT = S // 4         # pool-output rows per partition

F32 = mybir.dt.float32
F16 = mybir.dt.float16
ALU = mybir.AluOpType
AF = mybir.ActivationFunctionType

_cache: dict = {}

# Exposed for test harnesses: the BassKernelResults of the last run.
LAST_RESULT = None


def _build(w0: float, w1: float, w2: float, bias: float) -> bass.Bass:
    nc = bass.Bass()
    # x is host-padded with 3 rows at the front of L: padded row r holds
    # true row r-3 (see module docstring).
    x = nc.dram_tensor("x", [BPC, L + 3, D], F32, kind="ExternalInput")
    # wd holds the three 128x128 diagonal matrices w_k * I (fp16), used as
    # stationary operands so TensorE computes the conv taps as accumulating
    # "matmuls" (diag(w) @ X == w * X elementwise, partition-preserving).
    wd = nc.dram_tensor("wd", [128, 3 * 128], F16, kind="ExternalInput")
    y = nc.dram_tensor("y", [BPC, LP, D], F16, kind="ExternalOutput")

    xrow = D              # elements per L-row
    xbat = (L + 3) * D    # elements per (padded) input batch
    ybat = LP * D

    with TileContext(nc) as tc:
        with (
            tc.tile_pool(name="xp", bufs=2) as xp,
            tc.tile_pool(name="yp", bufs=2) as yp,
            tc.tile_pool(name="wp", bufs=1) as wp,
            tc.tile_pool(name="cp", bufs=2, space="PSUM") as cp,
            tc.tile_pool(name="pp", bufs=2) as pp,
            tc.tile_pool(name="rp", bufs=2) as rp,
        ):
            # The three stationary diag(w_k) matrices, loaded once up front
            # on the sync (HWDGE) queue so the SWDGE input stream is not
            # delayed.
            WD = wp.tile([128, 3 * 128], F16)
            nc.sync.dma_start(
                out=WD[:, :],
                in_=bass.AP(wd, 0, [[3 * 128, 128], [1, 3 * 128]]),
            )
            for b in range(BPC):
                last = b == BPC - 1
                # Input row-chunks, conv q-segments and pool j-segments are
                # aligned so each conv segment only needs already-landed
                # chunks (conv q taps local rows [2qa, 2qb+1)) and each pool
                # segment only needs finished conv rows (q in [2ja, 2jb+1)).
                # The last tile gets a finer tail so the final
                # load->conv->pool->store chain after the last chunk is short.
                if last:
                    # Only the 1-row pool seg (7,8) needs the final 2-row
                    # chunk (via conv q=16), keeping the post-stream tail to
                    # a single short FD=512 chain.
                    chunks = [(0, 5), (5, 13), (13, 21), (21, 29), (29, 33), (33, 35)]
                    poolsegs = [(0, 4), (4, 7), (7, 8)]
                else:
                    chunks = [(0, 5), (5, 13), (13, 21), (21, 29), (29, 35)]
                    poolsegs = [(0, 4), (4, 8)]
                convsegs = [(0, 4), (4, 8), (8, 12), (12, 16), (16, 17)]

                # Partition p holds padded rows [32p, 32p+35) = true rows
                # [32p-3, 32p+32): 3 halo rows, then its own 32 rows.
                X = xp.tile([128, (S + 3) * D], F16)
                Xv = X[:, :].rearrange("p (r d) -> p r d", d=D)
                Y = yp.tile([128, Q * D], F16)
                y3 = Y[:, :].rearrange("p (q d) -> p q d", d=D)
                P = pp.tile([128, JT * D], F16)
                p3 = P[:, :].rearrange("p (j d) -> p j d", d=D)
                R = rp.tile([128, JT * D], F16)

                for r0, r1 in chunks:
                    nc.gpsimd.dma_start(
                        out=X[:, r0 * D : r1 * D],
                        in_=bass.AP(
                            x,
                            b * xbat + r0 * xrow,
                            [[S * xrow, 128], [1, (r1 - r0) * xrow]],
                        ),
                    )

                # conv, shifted by +1 (the host subtracts 1 from the final
                # output): partition p's conv row q (local) is
                # c[16p - 1 + q] = w0*x[2q] + w1*x[2q+1] + w2*x[2q+2] + bias+1
                # (x indices local to the partition's 35-row strip).
                # The taps run on the otherwise-idle TensorE: diag(w_k) as
                # the stationary makes a matmul a partition-preserving
                # elementwise scale, and the three taps accumulate in a PSUM
                # bank (fp32). Matmuls are grouped by tap so the stationary
                # is swapped 3x per wave, not per row. ScalarE then evicts
                # PSUM -> fp16 SBUF, folding in bias+1 via the activation
                # bias. The global left pool pad (q=0 of partition 0, conv
                # over the 3 host pad rows) is forced very negative by
                # sign-based pad values chosen on the host, so no -inf
                # memset is needed.
                for qa, qb in convsegs:
                    nq = qb - qa
                    C4 = cp.tile([128, nq * 512], F32)
                    for k in range(3):
                        Wk = WD[:, k * 128 : (k + 1) * 128]
                        for q in range(qa, qb):
                            nc.tensor.matmul(
                                C4[:, (q - qa) * 512 : (q - qa + 1) * 512],
                                Wk,
                                Xv[:, 2 * q + k, :],
                                start=(k == 0),
                                stop=(k == 2),
                            )
                    nc.scalar.activation(
                        Y[:, qa * D : qb * D], C4[:, :], AF.Copy, bias=bias + 1.0
                    )

                # maxpool (pre-activation; ELU is monotonic), all shifted +1:
                # P[8p + j] = max(y3[2j], y3[2j+1], y3[2j+2]) = v + 1; then
                # out+1 = max(exp(min(v,0)), v+1), via one 4x dual-op
                # tensor_scalar (m = min(P-1, 0)), one ScalarE Exp, one 2x
                # tensor_tensor max. Stores go out fp16 on the sync (HWDGE)
                # queue, parallel to the SWDGE input queue.
                for ja, jb in poolsegs:
                    ps = p3[:, ja:jb, :]
                    pf = P[:, ja * D : jb * D]
                    rs = R[:, ja * D : jb * D]
                    nc.vector.tensor_tensor(
                        ps,
                        y3[:, 2 * ja : 2 * jb - 1 : 2, :],
                        y3[:, 2 * ja + 1 : 2 * jb : 2, :],
                        op=ALU.max,
                    )
                    nc.vector.tensor_tensor(
                        ps, ps, y3[:, 2 * ja + 2 : 2 * jb + 1 : 2, :], op=ALU.max
                    )
                    nc.vector.tensor_scalar(
                        rs, pf, -1.0, 0.0, op0=ALU.add, op1=ALU.min
                    )
                    nc.scalar.activation(rs, rs, AF.Exp)
                    nc.vector.tensor_tensor(rs, rs, pf, op=ALU.max)
                    nc.sync.dma_start(
                        out=bass.AP(
                            y,
                            b * ybat + ja * xrow,
                            [[JT * xrow, 128], [1, (jb - ja) * xrow]],
                        ),
                        in_=rs,
                    )
    return nc


def kernel(x: np.ndarray, w: np.ndarray, b: np.ndarray) -> np.ndarray:
    global LAST_RESULT
    w = np.asarray(w, dtype=np.float32)
    bb = np.asarray(b, dtype=np.float32)
    key = (float(w[0]), float(w[1]), float(w[2]), float(bb[0]))
    if key not in _cache:
        _cache[key] = _build(*key)
    nc = _cache[key]

    x = np.asarray(x, dtype=np.float32)
    assert x.shape == (B, L, D), x.shape
    xpad = np.zeros((B, L + 3, D), dtype=np.float32)
    xpad[:, 3:, :] = x
    # Pad rows 0/1 are chosen so the left pool pad c[-1] = w0*p0 + w1*p1 +
    # bias + 1 is hugely negative (it must lose every max against real conv
    # values; the reference excludes the pool pad via -inf). Row 2 stays 0:
    # it is the conv's own zero left-pad, used by c[0].
    w0, w1 = float(w[0]), float(w[1])
    C = min(60000.0, 40000.0 / max(abs(w0) + abs(w1), 1e-3))
    xpad[:, 0, :] = -np.sign(w0) * C if w0 != 0.0 else 0.0
    xpad[:, 1, :] = -np.sign(w1) * C if w1 != 0.0 else 0.0
    wdiag = np.concatenate(
        [np.eye(128, dtype=np.float16) * np.float16(w[k]) for k in range(3)],
        axis=1,
    )
    in_maps = [
        {
            "x": np.ascontiguousarray(xpad[c * BPC : (c + 1) * BPC]),
            "wd": wdiag,
        }
        for c in range(N_CORES)
    ]
    res = run_bass_kernel_spmd(nc, in_maps, core_ids=list(range(N_CORES)))
    LAST_RESULT = res
    out = np.concatenate([r["y"] for r in res.results], axis=0)
    # device computes out+1 in fp16 (see _build); undo the shift here
    return out.astype(np.float32) - 1.0


# revision 18
# speedup vs baseline: 1.1184x; 1.0565x over previous
"""Trainium2 Bass kernel for nn_DistillingLayer: per-channel shared-weight
Conv1d(k=3, stride=2, pad=1) + ELU + MaxPool1d(k=3, stride=2, pad=1) over
x:(16, 4096, 512) f32 -> out:(16, 1024, 512) f32.

Strategy (v2 — fp16 on-chip, DMA-roofline focused)
--------------------------------------------------
- Data-parallel over batch: 8 cores x 2 batches each. No communication.
- The kernel is HBM-bound (must read 16.8 MB + write out per core at
  ~358 GB/s/NC). v1 was jointly Vector- and DMA-limited (VectorE busy
  71 us, DMA 68 us, e2e 103 us). v2 computes in fp16 so every DVE op
  runs in the 2x_1P perf mode, halving VectorE time and leaving the
  input DMA stream as the only bottleneck.
- Layout: L lives in the SBUF free dimension. One tile per batch:
  each partition owns S=32 consecutive L-rows (x D=512 channels) plus a
  3-row halo loaded with overlap from HBM (9.4% overhead vs 18.75% for
  v1's S=16), so conv+pool stay per-partition local.
- The input is zero-padded by 3 L-rows on the host: uniform
  full-128-partition DMAs and free conv left-padding.
- Input DMAs run on the gpsimd (SWDGE) queue and cast f32->fp16 in the
  SDMA datapath. Each tile is split into 4-5 row-chunks so compute
  starts after the first chunk lands and the kernel tail is short.
- ELU is monotonic, so maxpool commutes: pool pre-activation conv
  outputs, ELU once on the pooled rows. Conv bias is folded into the
  first tap (ScalarE Copy activation applies scale+bias); taps 2/3 are
  VectorE scalar_tensor_tensor accumulates; pool is two VectorE max
  passes; ELU(v) = max(exp(min(v,0))-1, v) via two ScalarE activations
  + one fused VectorE scalar_tensor_tensor.
- Outputs are stored as fp16 via the sync (HWDGE) queue — it runs in
  parallel with the SWDGE input queue, so stores never block loads —
  and upcast to f32 on the host (absmax-scaled error ~1e-3, gate 2e-2).
- Weights/bias are baked as immediates; the compiled module is cached
  per (w, b) value.

Toolchain workaround (see inline comment): a BIR post-pass splits
multi-wait instructions — this walrus build allows one sync wait per
instruction.
"""

import json as _json
import os
import sys

import numpy as np

for _p in ("/opt/trn_rl_repo", "/root/.axon_site/_ro/trn_rl_repo"):
    if os.path.isdir(_p) and _p not in sys.path:
        sys.path.append(_p)

import concourse.bass as bass
import concourse.bass2jax as bass2jax
import concourse.bass_utils as bass_utils
import concourse.mybir as mybir
from concourse.bass_utils import run_bass_kernel_spmd
from concourse.tile import TileContext

# ---------------------------------------------------------------------------
# REQUIRED workaround: this container's walrus build rejects instructions
# carrying more than one sync wait ("Too many sync wait commands" in
# setupSyncWait). Tile's scheduler freely attaches several waits to one
# instruction, so post-process the BIR JSON before compile: hoist all but the
# last wait onto same-engine NoOps inserted just before the instruction
# (per-engine program order makes sequential waits equivalent to a
# multi-wait).
# ---------------------------------------------------------------------------

_orig_compile_bir_kernel = bass_utils.compile_bir_kernel


def _split_multi_waits(bir_json: bytes) -> bytes:
    j = _json.loads(bir_json)
    ctr = 0
    changed = False
    for fn in j["functions"]:
        for bb in fn["blocks"]:
            out = []
            for ins in bb["instructions"]:
                si = ins.get("sync_info")
                waits = (si.get("on_wait") or []) if si else []
                if len(waits) > 1:
                    changed = True
                    for w in waits[:-1]:
                        ctr += 1
                        out.append(
                            {
                                "debug": ins.get("debug", 0),
                                "engine": ins["engine"],
                                "ins": [],
                                "outs": [],
                                "name": f"waitsplit-{ctr}",
                                "opcode": "NoOp",
                                "text_hint": "waitsplit",
                                "sync_info": {"on_update": [], "on_wait": [w]},
                            }
                        )
                    si["on_wait"] = [waits[-1]]
                out.append(ins)
            bb["instructions"] = out
    if not changed:
        return bir_json
    return _json.dumps(j).encode()


def _patched_compile_bir_kernel(bir_json, tmpdir, neff_name="file.neff"):
    return _orig_compile_bir_kernel(_split_multi_waits(bir_json), tmpdir, neff_name)


bass_utils.compile_bir_kernel = _patched_compile_bir_kernel
bass2jax.compile_bir_kernel = _patched_compile_bir_kernel

# The first TileContext exit barrier's per-engine drains are redundant (the
# tail waits already cover all completions); use the cheap sequencer-level
# variant there. The SECOND barrier stays full — its drains restore
# engine/queue state so the loaded NEFF can re-execute.
try:
    from concourse.vector_clock import ScopedClock as _ScopedClock

    def _tail_drain_and_barrier(self, tick_clock, wait_clock):
        drain_inst = self.nc.sync.drain()
        wait_clock.add_sem_waits(
            drain_inst.ins, _ScopedClock({None: tick_clock.global_clock})
        )
        self.nc.all_engine_barrier(sem_only=True)
        assert self.sems is not None
        popped = self.nc._tile_sem_poison_stack.pop()
        assert popped is self._sem_poison
        self.nc.clear_and_free_semaphores(list(self.sems.allocated().values()))
        self.nc.all_engine_barrier()

    TileContext._drain_and_barrier = _tail_drain_and_barrier
except Exception:
    pass

# ---------------------------------------------------------------------------

N_CORES = 8
B, L, D = 16, 4096, 512
BPC = B // N_CORES  # batches per core
LC = L // 2         # conv output length
LP = LC // 2        # pool output length
S = 32              # input L-rows owned per partition (128 * 32 = 4096)
Q = S // 2 + 1      # conv rows per partition (incl. 1 left-halo conv row)
JT = S // 4         # pool-output rows per partition

F32 = mybir.dt.float32
F16 = mybir.dt.float16
ALU = mybir.AluOpType
AF = mybir.ActivationFunctionType

_cache: dict = {}

# Exposed for test harnesses: the BassKernelResults of the last run.
LAST_RESULT = None


def _build(w0: float, w1: float, w2: float, bias: float) -> bass.Bass:
    nc = bass.Bass()
    # x is host-padded with 3 rows at the front of L: padded row r holds
    # true row r-3 (see module docstring).
    x = nc.dram_tensor("x", [BPC, L + 3, D], F32, kind="ExternalInput")
    # wd holds the three 128x128 diagonal matrices w_k * I (fp16), used as
    # stationary operands so TensorE computes the conv taps as accumulating
    # "matmuls" (diag(w) @ X == w * X elementwise, partition-preserving).
    wd = nc.dram_tensor("wd", [128, 3 * 128], F16, kind="ExternalInput")
    y = nc.dram_tensor("y", [BPC, LP, D], F16, kind="ExternalOutput")

    xrow = D              # elements per L-row
    xbat = (L + 3) * D    # elements per (padded) input batch
    ybat = LP * D

    with TileContext(nc) as tc:
        with (
            tc.tile_pool(name="xp", bufs=2) as xp,
            tc.tile_pool(name="yp", bufs=2) as yp,
            tc.tile_pool(name="wp", bufs=1) as wp,
            tc.tile_pool(name="cp", bufs=2, space="PSUM") as cp,
            tc.tile_pool(name="pp", bufs=2) as pp,
            tc.tile_pool(name="rp", bufs=2) as rp,
        ):
            # The three stationary diag(w_k) matrices, loaded once up front
            # on the sync (HWDGE) queue so the SWDGE input stream is not
            # delayed.
            WD = wp.tile([128, 3 * 128], F16)
            nc.sync.dma_start(
                out=WD[:, :],
                in_=bass.AP(wd, 0, [[3 * 128, 128], [1, 3 * 128]]),
            )
            # Input row-chunks, conv q-waves and pool j-segments are aligned
            # so each conv wave only needs already-landed chunks (conv row q
            # taps local rows [2qa, 2qb+1)) and each pool segment only needs
            # finished conv rows (q in [2ja, 2jb+1)). The two batch tiles'
            # chunks are INTERLEAVED in the SWDGE stream, so each tile's
            # compute spreads across the whole stream and only one short
            # wave+pool chain hangs off the final chunk (tile 1's pool seg
            # (7,8) via conv q=16).
            chunks = [(0, 5), (5, 13), (13, 21), (21, 29), (29, 33), (33, 35)]
            convsegs = [(0, 4), (4, 8), (8, 12), (12, 16), (16, 17)]
            poolsegs = [(0, 4), (4, 7), (7, 8)]

            tiles = []
            for b in range(BPC):
                # Partition p holds padded rows [32p, 32p+35) = true rows
                # [32p-3, 32p+32): 3 halo rows, then its own 32 rows.
                X = xp.tile([128, (S + 3) * D], F16)
                Y = yp.tile([128, Q * D], F16)
                P = pp.tile([128, JT * D], F16)
                R = rp.tile([128, JT * D], F16)
                tiles.append((b, X, Y, P, R))

            for ci in range(len(chunks)):
                r0, r1 = chunks[ci]
                for b, X, Y, P, R in tiles:
                    nc.gpsimd.dma_start(
                        out=X[:, r0 * D : r1 * D],
                        in_=bass.AP(
                            x,
                            b * xbat + r0 * xrow,
                            [[S * xrow, 128], [1, (r1 - r0) * xrow]],
                        ),
                    )

            # conv, shifted by +1 (the host subtracts 1 from the final
            # output): partition p's conv row q (local) is
            # c[16p - 1 + q] = w0*x[2q] + w1*x[2q+1] + w2*x[2q+2] + bias+1
            # (x indices local to the partition's 35-row strip).
            # The taps run on the otherwise-idle TensorE: diag(w_k) as the
            # stationary makes a matmul a partition-preserving elementwise
            # scale, and the three taps accumulate in a PSUM bank (fp32).
            # Matmuls are grouped by tap so the stationary is swapped 3x per
            # wave, not per row. ScalarE then evicts PSUM -> fp16 SBUF,
            # folding in bias+1 via the activation bias. The global left
            # pool pad (q=0 of partition 0, conv over the 3 host pad rows)
            # is forced very negative by sign-based pad values chosen on the
            # host, so no -inf memset is needed.
            for qa, qb in convsegs:
                nq = qb - qa
                for b, X, Y, P, R in tiles:
                    Xv = X[:, :].rearrange("p (r d) -> p r d", d=D)
                    C4 = cp.tile([128, nq * 512], F32)
                    for k in range(3):
                        Wk = WD[:, k * 128 : (k + 1) * 128]
                        for q in range(qa, qb):
                            nc.tensor.matmul(
                                C4[:, (q - qa) * 512 : (q - qa + 1) * 512],
                                Wk,
                                Xv[:, 2 * q + k, :],
                                start=(k == 0),
                                stop=(k == 2),
                            )
                    nc.scalar.activation(
                        Y[:, qa * D : qb * D], C4[:, :], AF.Copy, bias=bias + 1.0
                    )

            # maxpool (pre-activation; ELU is monotonic), all shifted +1:
            # P[8p + j] = max(y3[2j], y3[2j+1], y3[2j+2]) = v + 1; then
            # out+1 = max(exp(min(v,0)), v+1), via one 4x dual-op
            # tensor_scalar (m = min(P-1, 0)), one ScalarE Exp, one 2x
            # tensor_tensor max. Stores go out fp16 on the sync (HWDGE)
            # queue, parallel to the SWDGE input queue.
            for ja, jb in poolsegs:
                for b, X, Y, P, R in tiles:
                    y3 = Y[:, :].rearrange("p (q d) -> p q d", d=D)
                    p3 = P[:, :].rearrange("p (j d) -> p j d", d=D)
                    ps = p3[:, ja:jb, :]
                    pf = P[:, ja * D : jb * D]
                    rs = R[:, ja * D : jb * D]
                    nc.vector.tensor_tensor(
                        ps,
                        y3[:, 2 * ja : 2 * jb - 1 : 2, :],
                        y3[:, 2 * ja + 1 : 2 * jb : 2, :],
                        op=ALU.max,
                    )
                    nc.vector.tensor_tensor(
                        ps, ps, y3[:, 2 * ja + 2 : 2 * jb + 1 : 2, :], op=ALU.max
                    )
                    nc.vector.tensor_scalar(
                        rs, pf, -1.0, 0.0, op0=ALU.add, op1=ALU.min
                    )
                    nc.scalar.activation(rs, rs, AF.Exp)
                    nc.vector.tensor_tensor(rs, rs, pf, op=ALU.max)
                    nc.sync.dma_start(
                        out=bass.AP(
                            y,
                            b * ybat + ja * xrow,
                            [[JT * xrow, 128], [1, (jb - ja) * xrow]],
                        ),
                        in_=rs,
                    )
    return nc


def kernel(x: np.ndarray, w: np.ndarray, b: np.ndarray) -> np.ndarray:
    global LAST_RESULT
    w = np.asarray(w, dtype=np.float32)
    bb = np.asarray(b, dtype=np.float32)
    key = (float(w[0]), float(w[1]), float(w[2]), float(bb[0]))
    if key not in _cache:
        _cache[key] = _build(*key)
    nc = _cache[key]

    x = np.asarray(x, dtype=np.float32)
    assert x.shape == (B, L, D), x.shape
    xpad = np.zeros((B, L + 3, D), dtype=np.float32)
    xpad[:, 3:, :] = x
    # Pad rows 0/1 are chosen so the left pool pad c[-1] = w0*p0 + w1*p1 +
    # bias + 1 is hugely negative (it must lose every max against real conv
    # values; the reference excludes the pool pad via -inf). Row 2 stays 0:
    # it is the conv's own zero left-pad, used by c[0].
    w0, w1 = float(w[0]), float(w[1])
    C = min(60000.0, 40000.0 / max(abs(w0) + abs(w1), 1e-3))
    xpad[:, 0, :] = -np.sign(w0) * C if w0 != 0.0 else 0.0
    xpad[:, 1, :] = -np.sign(w1) * C if w1 != 0.0 else 0.0
    wdiag = np.concatenate(
        [np.eye(128, dtype=np.float16) * np.float16(w[k]) for k in range(3)],
        axis=1,
    )
    in_maps = [
        {
            "x": np.ascontiguousarray(xpad[c * BPC : (c + 1) * BPC]),
            "wd": wdiag,
        }
        for c in range(N_CORES)
    ]
    res = run_bass_kernel_spmd(nc, in_maps, core_ids=list(range(N_CORES)))
    LAST_RESULT = res
    out = np.concatenate([r["y"] for r in res.results], axis=0)
    # device computes out+1 in fp16 (see _build); undo the shift here
    return out.astype(np.float32) - 1.0


# revision 19
# speedup vs baseline: 1.1252x; 1.0061x over previous
"""Trainium2 Bass kernel for nn_DistillingLayer: per-channel shared-weight
Conv1d(k=3, stride=2, pad=1) + ELU + MaxPool1d(k=3, stride=2, pad=1) over
x:(16, 4096, 512) f32 -> out:(16, 1024, 512) f32.

Strategy (v2 — fp16 on-chip, DMA-roofline focused)
--------------------------------------------------
- Data-parallel over batch: 8 cores x 2 batches each. No communication.
- The kernel is HBM-bound (must read 16.8 MB + write out per core at
  ~358 GB/s/NC). v1 was jointly Vector- and DMA-limited (VectorE busy
  71 us, DMA 68 us, e2e 103 us). v2 computes in fp16 so every DVE op
  runs in the 2x_1P perf mode, halving VectorE time and leaving the
  input DMA stream as the only bottleneck.
- Layout: L lives in the SBUF free dimension. One tile per batch:
  each partition owns S=32 consecutive L-rows (x D=512 channels) plus a
  3-row halo loaded with overlap from HBM (9.4% overhead vs 18.75% for
  v1's S=16), so conv+pool stay per-partition local.
- The input is zero-padded by 3 L-rows on the host: uniform
  full-128-partition DMAs and free conv left-padding.
- Input DMAs run on the gpsimd (SWDGE) queue and cast f32->fp16 in the
  SDMA datapath. Each tile is split into 4-5 row-chunks so compute
  starts after the first chunk lands and the kernel tail is short.
- ELU is monotonic, so maxpool commutes: pool pre-activation conv
  outputs, ELU once on the pooled rows. Conv bias is folded into the
  first tap (ScalarE Copy activation applies scale+bias); taps 2/3 are
  VectorE scalar_tensor_tensor accumulates; pool is two VectorE max
  passes; ELU(v) = max(exp(min(v,0))-1, v) via two ScalarE activations
  + one fused VectorE scalar_tensor_tensor.
- Outputs are stored as fp16 via the sync (HWDGE) queue — it runs in
  parallel with the SWDGE input queue, so stores never block loads —
  and upcast to f32 on the host (absmax-scaled error ~1e-3, gate 2e-2).
- Weights/bias are baked as immediates; the compiled module is cached
  per (w, b) value.

Toolchain workaround (see inline comment): a BIR post-pass splits
multi-wait instructions — this walrus build allows one sync wait per
instruction.
"""

import json as _json
import os
import sys

import numpy as np

for _p in ("/opt/trn_rl_repo", "/root/.axon_site/_ro/trn_rl_repo"):
    if os.path.isdir(_p) and _p not in sys.path:
        sys.path.append(_p)

import concourse.bass as bass
import concourse.bass2jax as bass2jax
import concourse.bass_utils as bass_utils
import concourse.mybir as mybir
from concourse.bass_utils import run_bass_kernel_spmd
from concourse.tile import TileContext

# ---------------------------------------------------------------------------
# REQUIRED workaround: this container's walrus build rejects instructions
# carrying more than one sync wait ("Too many sync wait commands" in
# setupSyncWait). Tile's scheduler freely attaches several waits to one
# instruction, so post-process the BIR JSON before compile: hoist all but the
# last wait onto same-engine NoOps inserted just before the instruction
# (per-engine program order makes sequential waits equivalent to a
# multi-wait).
# ---------------------------------------------------------------------------

_orig_compile_bir_kernel = bass_utils.compile_bir_kernel


def _split_multi_waits(bir_json: bytes) -> bytes:
    j = _json.loads(bir_json)
    ctr = 0
    changed = False
    for fn in j["functions"]:
        for bb in fn["blocks"]:
            out = []
            for ins in bb["instructions"]:
                si = ins.get("sync_info")
                waits = (si.get("on_wait") or []) if si else []
                if len(waits) > 1:
                    changed = True
                    for w in waits[:-1]:
                        ctr += 1
                        out.append(
                            {
                                "debug": ins.get("debug", 0),
                                "engine": ins["engine"],
                                "ins": [],
                                "outs": [],
                                "name": f"waitsplit-{ctr}",
                                "opcode": "NoOp",
                                "text_hint": "waitsplit",
                                "sync_info": {"on_update": [], "on_wait": [w]},
                            }
                        )
                    si["on_wait"] = [waits[-1]]
                out.append(ins)
            bb["instructions"] = out
    if not changed:
        return bir_json
    return _json.dumps(j).encode()


def _patched_compile_bir_kernel(bir_json, tmpdir, neff_name="file.neff"):
    return _orig_compile_bir_kernel(_split_multi_waits(bir_json), tmpdir, neff_name)


bass_utils.compile_bir_kernel = _patched_compile_bir_kernel
bass2jax.compile_bir_kernel = _patched_compile_bir_kernel

# The first TileContext exit barrier's per-engine drains are redundant (the
# tail waits already cover all completions); use the cheap sequencer-level
# variant there. The SECOND barrier stays full — its drains restore
# engine/queue state so the loaded NEFF can re-execute.
try:
    from concourse.vector_clock import ScopedClock as _ScopedClock

    def _tail_drain_and_barrier(self, tick_clock, wait_clock):
        drain_inst = self.nc.sync.drain()
        wait_clock.add_sem_waits(
            drain_inst.ins, _ScopedClock({None: tick_clock.global_clock})
        )
        self.nc.all_engine_barrier(sem_only=True)
        assert self.sems is not None
        popped = self.nc._tile_sem_poison_stack.pop()
        assert popped is self._sem_poison
        self.nc.clear_and_free_semaphores(list(self.sems.allocated().values()))
        self.nc.all_engine_barrier()

    TileContext._drain_and_barrier = _tail_drain_and_barrier
except Exception:
    pass

# ---------------------------------------------------------------------------

N_CORES = 8
B, L, D = 16, 4096, 512
BPC = B // N_CORES  # batches per core
LC = L // 2         # conv output length
LP = LC // 2        # pool output length
S = 32              # input L-rows owned per partition (128 * 32 = 4096)
Q = S // 2 + 1      # conv rows per partition (incl. 1 left-halo conv row)
JT = S // 4         # pool-output rows per partition

F32 = mybir.dt.float32
F16 = mybir.dt.float16
ALU = mybir.AluOpType
AF = mybir.ActivationFunctionType

_cache: dict = {}

# Exposed for test harnesses: the BassKernelResults of the last run.
LAST_RESULT = None


def _build(w0: float, w1: float, w2: float, bias: float) -> bass.Bass:
    nc = bass.Bass()
    # x is host-padded with 3 rows at the front of L: padded row r holds
    # true row r-3 (see module docstring).
    x = nc.dram_tensor("x", [BPC, L + 3, D], F32, kind="ExternalInput")
    # wd holds the three 128x128 diagonal matrices w_k * I (fp16), used as
    # stationary operands so TensorE computes the conv taps as accumulating
    # "matmuls" (diag(w) @ X == w * X elementwise, partition-preserving).
    wd = nc.dram_tensor("wd", [128, 3 * 128], F16, kind="ExternalInput")
    y = nc.dram_tensor("y", [BPC, LP, D], F16, kind="ExternalOutput")

    xrow = D              # elements per L-row
    xbat = (L + 3) * D    # elements per (padded) input batch
    ybat = LP * D

    with TileContext(nc) as tc:
        with (
            tc.tile_pool(name="xp", bufs=2) as xp,
            tc.tile_pool(name="yp", bufs=2) as yp,
            tc.tile_pool(name="wp", bufs=1) as wp,
            tc.tile_pool(name="cp", bufs=2, space="PSUM") as cp,
            tc.tile_pool(name="pp", bufs=2) as pp,
            tc.tile_pool(name="rp", bufs=2) as rp,
        ):
            # The three stationary diag(w_k) matrices, loaded once up front
            # on the sync (HWDGE) queue so the SWDGE input stream is not
            # delayed.
            WD = wp.tile([128, 3 * 128], F16)
            nc.sync.dma_start(
                out=WD[:, :],
                in_=bass.AP(wd, 0, [[3 * 128, 128], [1, 3 * 128]]),
            )
            # Input row-chunks, conv q-waves and pool j-segments are aligned
            # so each conv wave only needs already-landed chunks (conv row q
            # taps local rows [2qa, 2qb+1)) and each pool segment only needs
            # finished conv rows (q in [2ja, 2jb+1)). The two batch tiles'
            # chunks are INTERLEAVED in the SWDGE stream, so each tile's
            # compute spreads across the whole stream and only one short
            # wave+pool chain hangs off the final chunk (tile 1's pool seg
            # (7,8) via conv q=16).
            # Chunk c delivers exactly the rows conv wave c needs (wave
            # (qa,qb) taps rows [2qa, 2qb]), and waves/pool segs shrink
            # toward the stream end so the chain hanging off the final chunk
            # is one 3-matmul + FD-512 evict/pool/store sequence.
            chunks = [(0, 9), (9, 17), (17, 25), (25, 29), (29, 33), (33, 35)]
            convsegs = [(0, 4), (4, 8), (8, 12), (12, 14), (14, 16), (16, 17)]
            poolsegs = [(0, 4), (4, 6), (6, 7), (7, 8)]

            tiles = []
            for b in range(BPC):
                # Partition p holds padded rows [32p, 32p+35) = true rows
                # [32p-3, 32p+32): 3 halo rows, then its own 32 rows.
                X = xp.tile([128, (S + 3) * D], F16)
                Y = yp.tile([128, Q * D], F16)
                P = pp.tile([128, JT * D], F16)
                R = rp.tile([128, JT * D], F16)
                tiles.append((b, X, Y, P, R))

            for ci in range(len(chunks)):
                r0, r1 = chunks[ci]
                for b, X, Y, P, R in tiles:
                    nc.gpsimd.dma_start(
                        out=X[:, r0 * D : r1 * D],
                        in_=bass.AP(
                            x,
                            b * xbat + r0 * xrow,
                            [[S * xrow, 128], [1, (r1 - r0) * xrow]],
                        ),
                    )

            # conv, shifted by +1 (the host subtracts 1 from the final
            # output): partition p's conv row q (local) is
            # c[16p - 1 + q] = w0*x[2q] + w1*x[2q+1] + w2*x[2q+2] + bias+1
            # (x indices local to the partition's 35-row strip).
            # The taps run on the otherwise-idle TensorE: diag(w_k) as the
            # stationary makes a matmul a partition-preserving elementwise
            # scale, and the three taps accumulate in a PSUM bank (fp32).
            # Matmuls are grouped by tap so the stationary is swapped 3x per
            # wave, not per row. ScalarE then evicts PSUM -> fp16 SBUF,
            # folding in bias+1 via the activation bias. The global left
            # pool pad (q=0 of partition 0, conv over the 3 host pad rows)
            # is forced very negative by sign-based pad values chosen on the
            # host, so no -inf memset is needed.
            for qa, qb in convsegs:
                nq = qb - qa
                for b, X, Y, P, R in tiles:
                    Xv = X[:, :].rearrange("p (r d) -> p r d", d=D)
                    C4 = cp.tile([128, nq * 512], F32)
                    for k in range(3):
                        Wk = WD[:, k * 128 : (k + 1) * 128]
                        for q in range(qa, qb):
                            nc.tensor.matmul(
                                C4[:, (q - qa) * 512 : (q - qa + 1) * 512],
                                Wk,
                                Xv[:, 2 * q + k, :],
                                start=(k == 0),
                                stop=(k == 2),
                            )
                    nc.scalar.activation(
                        Y[:, qa * D : qb * D], C4[:, :], AF.Copy, bias=bias + 1.0
                    )

            # maxpool (pre-activation; ELU is monotonic), all shifted +1:
            # P[8p + j] = max(y3[2j], y3[2j+1], y3[2j+2]) = v + 1; then
            # out+1 = max(exp(min(v,0)), v+1), via one 4x dual-op
            # tensor_scalar (m = min(P-1, 0)), one ScalarE Exp, one 2x
            # tensor_tensor max. Stores go out fp16 on the sync (HWDGE)
            # queue, parallel to the SWDGE input queue.
            for ja, jb in poolsegs:
                for b, X, Y, P, R in tiles:
                    y3 = Y[:, :].rearrange("p (q d) -> p q d", d=D)
                    p3 = P[:, :].rearrange("p (j d) -> p j d", d=D)
                    ps = p3[:, ja:jb, :]
                    pf = P[:, ja * D : jb * D]
                    rs = R[:, ja * D : jb * D]
                    nc.vector.tensor_tensor(
                        ps,
                        y3[:, 2 * ja : 2 * jb - 1 : 2, :],
                        y3[:, 2 * ja + 1 : 2 * jb : 2, :],
                        op=ALU.max,
                    )
                    nc.vector.tensor_tensor(
                        ps, ps, y3[:, 2 * ja + 2 : 2 * jb + 1 : 2, :], op=ALU.max
                    )
                    nc.vector.tensor_scalar(
                        rs, pf, -1.0, 0.0, op0=ALU.add, op1=ALU.min
                    )
                    nc.scalar.activation(rs, rs, AF.Exp)
                    nc.vector.tensor_tensor(rs, rs, pf, op=ALU.max)
                    nc.sync.dma_start(
                        out=bass.AP(
                            y,
                            b * ybat + ja * xrow,
                            [[JT * xrow, 128], [1, (jb - ja) * xrow]],
                        ),
                        in_=rs,
                    )
    return nc


def kernel(x: np.ndarray, w: np.ndarray, b: np.ndarray) -> np.ndarray:
    global LAST_RESULT
    w = np.asarray(w, dtype=np.float32)
    bb = np.asarray(b, dtype=np.float32)
    key = (float(w[0]), float(w[1]), float(w[2]), float(bb[0]))
    if key not in _cache:
        _cache[key] = _build(*key)
    nc = _cache[key]

    x = np.asarray(x, dtype=np.float32)
    assert x.shape == (B, L, D), x.shape
    xpad = np.zeros((B, L + 3, D), dtype=np.float32)
    xpad[:, 3:, :] = x
    # Pad rows 0/1 are chosen so the left pool pad c[-1] = w0*p0 + w1*p1 +
    # bias + 1 is hugely negative (it must lose every max against real conv
    # values; the reference excludes the pool pad via -inf). Row 2 stays 0:
    # it is the conv's own zero left-pad, used by c[0].
    w0, w1 = float(w[0]), float(w[1])
    C = min(60000.0, 40000.0 / max(abs(w0) + abs(w1), 1e-3))
    xpad[:, 0, :] = -np.sign(w0) * C if w0 != 0.0 else 0.0
    xpad[:, 1, :] = -np.sign(w1) * C if w1 != 0.0 else 0.0
    wdiag = np.concatenate(
        [np.eye(128, dtype=np.float16) * np.float16(w[k]) for k in range(3)],
        axis=1,
    )
    in_maps = [
        {
            "x": np.ascontiguousarray(xpad[c * BPC : (c + 1) * BPC]),
            "wd": wdiag,
        }
        for c in range(N_CORES)
    ]
    res = run_bass_kernel_spmd(nc, in_maps, core_ids=list(range(N_CORES)))
    LAST_RESULT = res
    out = np.concatenate([r["y"] for r in res.results], axis=0)
    # device computes out+1 in fp16 (see _build); undo the shift here
    return out.astype(np.float32) - 1.0


# revision 26
# speedup vs baseline: 1.1469x; 1.0193x over previous
"""Trainium2 Bass kernel for nn_DistillingLayer: per-channel shared-weight
Conv1d(k=3, stride=2, pad=1) + ELU + MaxPool1d(k=3, stride=2, pad=1) over
x:(16, 4096, 512) f32 -> out:(16, 1024, 512) f32.

Strategy (v2 — fp16 on-chip, DMA-roofline focused)
--------------------------------------------------
- Data-parallel over batch: 8 cores x 2 batches each. No communication.
- The kernel is HBM-bound (must read 16.8 MB + write out per core at
  ~358 GB/s/NC). v1 was jointly Vector- and DMA-limited (VectorE busy
  71 us, DMA 68 us, e2e 103 us). v2 computes in fp16 so every DVE op
  runs in the 2x_1P perf mode, halving VectorE time and leaving the
  input DMA stream as the only bottleneck.
- Layout: L lives in the SBUF free dimension. One tile per batch:
  each partition owns S=32 consecutive L-rows (x D=512 channels) plus a
  3-row halo loaded with overlap from HBM (9.4% overhead vs 18.75% for
  v1's S=16), so conv+pool stay per-partition local.
- The input is zero-padded by 3 L-rows on the host: uniform
  full-128-partition DMAs and free conv left-padding.
- Input DMAs run on the gpsimd (SWDGE) queue and cast f32->fp16 in the
  SDMA datapath. Each tile is split into 4-5 row-chunks so compute
  starts after the first chunk lands and the kernel tail is short.
- ELU is monotonic, so maxpool commutes: pool pre-activation conv
  outputs, ELU once on the pooled rows. Conv bias is folded into the
  first tap (ScalarE Copy activation applies scale+bias); taps 2/3 are
  VectorE scalar_tensor_tensor accumulates; pool is two VectorE max
  passes; ELU(v) = max(exp(min(v,0))-1, v) via two ScalarE activations
  + one fused VectorE scalar_tensor_tensor.
- Outputs are stored as fp16 via the sync (HWDGE) queue — it runs in
  parallel with the SWDGE input queue, so stores never block loads —
  and upcast to f32 on the host (absmax-scaled error ~1e-3, gate 2e-2).
- Weights/bias are baked as immediates; the compiled module is cached
  per (w, b) value.

Toolchain workaround (see inline comment): a BIR post-pass splits
multi-wait instructions — this walrus build allows one sync wait per
instruction.
"""

import json as _json
import os
import sys

import numpy as np

for _p in ("/opt/trn_rl_repo", "/root/.axon_site/_ro/trn_rl_repo"):
    if os.path.isdir(_p) and _p not in sys.path:
        sys.path.append(_p)

import concourse.bass as bass
import concourse.bass2jax as bass2jax
import concourse.bass_utils as bass_utils
import concourse.mybir as mybir
from concourse.bass_utils import run_bass_kernel_spmd
from concourse.tile import TileContext

# ---------------------------------------------------------------------------
# REQUIRED workaround: this container's walrus build rejects instructions
# carrying more than one sync wait ("Too many sync wait commands" in
# setupSyncWait). Tile's scheduler freely attaches several waits to one
# instruction, so post-process the BIR JSON before compile: hoist all but the
# last wait onto same-engine NoOps inserted just before the instruction
# (per-engine program order makes sequential waits equivalent to a
# multi-wait).
# ---------------------------------------------------------------------------

_orig_compile_bir_kernel = bass_utils.compile_bir_kernel


def _split_multi_waits(bir_json: bytes) -> bytes:
    j = _json.loads(bir_json)
    ctr = 0
    changed = False
    for fn in j["functions"]:
        for bb in fn["blocks"]:
            out = []
            for ins in bb["instructions"]:
                si = ins.get("sync_info")
                waits = (si.get("on_wait") or []) if si else []
                if len(waits) > 1:
                    changed = True
                    for w in waits[:-1]:
                        ctr += 1
                        out.append(
                            {
                                "debug": ins.get("debug", 0),
                                "engine": ins["engine"],
                                "ins": [],
                                "outs": [],
                                "name": f"waitsplit-{ctr}",
                                "opcode": "NoOp",
                                "text_hint": "waitsplit",
                                "sync_info": {"on_update": [], "on_wait": [w]},
                            }
                        )
                    si["on_wait"] = [waits[-1]]
                out.append(ins)
            bb["instructions"] = out
    if not changed:
        return bir_json
    return _json.dumps(j).encode()


def _patched_compile_bir_kernel(bir_json, tmpdir, neff_name="file.neff"):
    return _orig_compile_bir_kernel(_split_multi_waits(bir_json), tmpdir, neff_name)


bass_utils.compile_bir_kernel = _patched_compile_bir_kernel
bass2jax.compile_bir_kernel = _patched_compile_bir_kernel

# The first TileContext exit barrier's per-engine drains are redundant (the
# tail waits already cover all completions); use the cheap sequencer-level
# variant there. The SECOND barrier stays full — its drains restore
# engine/queue state so the loaded NEFF can re-execute.
try:
    from concourse.vector_clock import ScopedClock as _ScopedClock

    def _tail_drain_and_barrier(self, tick_clock, wait_clock):
        drain_inst = self.nc.sync.drain()
        wait_clock.add_sem_waits(
            drain_inst.ins, _ScopedClock({None: tick_clock.global_clock})
        )
        self.nc.all_engine_barrier(sem_only=True)
        assert self.sems is not None
        popped = self.nc._tile_sem_poison_stack.pop()
        assert popped is self._sem_poison
        self.nc.clear_and_free_semaphores(list(self.sems.allocated().values()))
        self.nc.all_engine_barrier(sem_only=True)

    TileContext._drain_and_barrier = _tail_drain_and_barrier
except Exception:
    pass

# ---------------------------------------------------------------------------

N_CORES = 8
B, L, D = 16, 4096, 512
BPC = B // N_CORES  # batches per core
LC = L // 2         # conv output length
LP = LC // 2        # pool output length
S = 32              # input L-rows owned per partition (128 * 32 = 4096)
Q = S // 2 + 1      # conv rows per partition (incl. 1 left-halo conv row)
JT = S // 4         # pool-output rows per partition

F32 = mybir.dt.float32
F16 = mybir.dt.float16
ALU = mybir.AluOpType
AF = mybir.ActivationFunctionType

_cache: dict = {}

# Exposed for test harnesses: the BassKernelResults of the last run.
LAST_RESULT = None


def _build(w0: float, w1: float, w2: float, bias: float) -> bass.Bass:
    nc = bass.Bass()
    # x is the raw unpadded input: partition p of a tile owns exactly rows
    # [32p, 32p+32) -- no halo rows are loaded (see the shift matrices).
    x = nc.dram_tensor("x", [BPC, L, D], F32, kind="ExternalInput")
    # wd holds six 128x128 stationary matrices (fp16): w_k * I for k=0..2
    # (diag(w) @ X == w * X elementwise, partition-preserving) and
    # w_k * eye(k=1), whose matmul routes partition p-1's row to partition
    # p -- used for the two boundary conv rows whose taps live in the
    # previous partition, replacing a 3-row halo reload from HBM.
    wd = nc.dram_tensor("wd", [128, 6 * 128], F16, kind="ExternalInput")
    y = nc.dram_tensor("y", [BPC, LP, D], F16, kind="ExternalOutput")

    xrow = D              # elements per L-row
    xbat = L * D          # elements per input batch
    ybat = LP * D

    with TileContext(nc) as tc:
        with (
            tc.tile_pool(name="xp", bufs=2) as xp,
            tc.tile_pool(name="yp", bufs=2) as yp,
            tc.tile_pool(name="wp", bufs=1) as wp,
            tc.tile_pool(name="cp", bufs=2, space="PSUM") as cp,
            tc.tile_pool(name="pp", bufs=2) as pp,
            tc.tile_pool(name="rp", bufs=2) as rp,
        ):
            # The six stationary matrices, loaded once up front on the sync
            # (HWDGE) queue so the SWDGE input stream is not delayed.
            WD = wp.tile([128, 6 * 128], F16)
            nc.sync.dma_start(
                out=WD[:, :],
                in_=bass.AP(wd, 0, [[6 * 128, 128], [1, 6 * 128]]),
            )
            # Input row-chunks, conv q-waves and pool j-segments are aligned
            # so each conv wave only needs already-landed chunks (conv row q
            # taps local rows [2qa, 2qb+1)) and each pool segment only needs
            # finished conv rows (q in [2ja, 2jb+1)). The two batch tiles'
            # chunks are INTERLEAVED in the SWDGE stream, so each tile's
            # compute spreads across the whole stream and only one short
            # wave+pool chain hangs off the final chunk (tile 1's pool seg
            # (7,8) via conv q=16).
            # Chunk c delivers exactly the rows conv wave c needs (wave
            # (qa,qb) with qa>=2 taps local rows [2qa-3, 2qb-2)), and
            # waves/pool segs shrink toward the stream end. The boundary
            # wave (0,2) runs last: its taps are the previous partition's
            # rows 29-31, routed cross-partition by the shift stationaries,
            # so it is gated by the final 1-row chunk like wave (16,17).
            chunks = [(0, 11), (11, 19), (19, 27), (27, 31), (31, 32)]
            convsegs = [(2, 6), (6, 10), (10, 14), (14, 16), (16, 17)]
            poolsegs = [(1, 4), (4, 6), (6, 7), (7, 8), (0, 1)]

            tiles = []
            for b in range(BPC):
                # Partition p holds exactly its own rows [32p, 32p+32).
                X = xp.tile([128, S * D], F16)
                Y = yp.tile([128, Q * D], F16)
                P = pp.tile([128, JT * D], F16)
                R = rp.tile([128, JT * D], F16)
                tiles.append((b, X, Y, P, R))

            for ci in range(len(chunks)):
                r0, r1 = chunks[ci]
                for b, X, Y, P, R in tiles:
                    nc.gpsimd.dma_start(
                        out=X[:, r0 * D : r1 * D],
                        in_=bass.AP(
                            x,
                            b * xbat + r0 * xrow,
                            [[S * xrow, 128], [1, (r1 - r0) * xrow]],
                        ),
                    )

            # conv, shifted by +1 (the host subtracts 1 from the final
            # output): partition p's conv row q (local) is
            # c[16p - 1 + q] = w0*x[2q] + w1*x[2q+1] + w2*x[2q+2] + bias+1
            # (x indices local to the partition's 35-row strip).
            # The taps run on the otherwise-idle TensorE: diag(w_k) as the
            # stationary makes a matmul a partition-preserving elementwise
            # scale, and the three taps accumulate in a PSUM bank (fp32).
            # Matmuls are grouped by tap so the stationary is swapped 3x per
            # wave, not per row. ScalarE then evicts PSUM -> fp16 SBUF,
            # folding in bias+1 via the activation bias. The global left
            # pool pad (q=0 of partition 0, conv over the 3 host pad rows)
            # is forced very negative by sign-based pad values chosen on the
            # host, so no -inf memset is needed.
            for qa, qb in convsegs:
                nq = qb - qa
                for b, X, Y, P, R in tiles:
                    Xv = X[:, :].rearrange("p (r d) -> p r d", d=D)
                    C4 = cp.tile([128, nq * 512], F32, tag="cw")
                    for k in range(3):
                        Wk = WD[:, k * 128 : (k + 1) * 128]
                        for q in range(qa, qb):
                            nc.tensor.matmul(
                                C4[:, (q - qa) * 512 : (q - qa + 1) * 512],
                                Wk,
                                Xv[:, 2 * q - 3 + k, :],
                                start=(k == 0),
                                stop=(k == 2),
                            )
                    nc.scalar.activation(
                        Y[:, qa * D : qb * D], C4[:, :], AF.Copy, bias=bias + 1.0
                    )

            # Boundary wave (0,2): conv rows q=0,1 of partition p tap rows
            # 29-31 of partition p-1 (routed by the shift stationaries
            # WD[:, (3+k)*128:]) plus local rows 0,1. Partition 0's q=0 gets
            # all-zero shift input (= the pool's excluded left pad), so it
            # is overwritten with -inf after eviction; its q=1 correctly
            # sees zero for the conv's left pad x[-1].
            for b, X, Y, P, R in tiles:
                Xv = X[:, :].rearrange("p (r d) -> p r d", d=D)
                C2 = cp.tile([128, 2 * 512], F32, tag="cw")
                for k in range(3):
                    Sk = WD[:, (3 + k) * 128 : (4 + k) * 128]
                    nc.tensor.matmul(
                        C2[:, 0:512],
                        Sk,
                        Xv[:, 29 + k, :],
                        start=(k == 0),
                        stop=(k == 2),
                    )
                nc.tensor.matmul(
                    C2[:, 512:1024], WD[:, 3 * 128 : 4 * 128], Xv[:, 31, :],
                    start=True, stop=False,
                )
                nc.tensor.matmul(
                    C2[:, 512:1024], WD[:, 1 * 128 : 2 * 128], Xv[:, 0, :],
                    start=False, stop=False,
                )
                nc.tensor.matmul(
                    C2[:, 512:1024], WD[:, 2 * 128 : 3 * 128], Xv[:, 1, :],
                    start=False, stop=True,
                )
                nc.scalar.activation(
                    Y[:, 0 : 2 * D], C2[:, :], AF.Copy, bias=bias + 1.0
                )
                nc.vector.memset(Y[0:1, 0:D], float("-inf"))

            # maxpool (pre-activation; ELU is monotonic), all shifted +1:
            # P[8p + j] = max(y3[2j], y3[2j+1], y3[2j+2]) = v + 1; then
            # out+1 = max(exp(min(v,0)), v+1), via one 4x dual-op
            # tensor_scalar (m = min(P-1, 0)), one ScalarE Exp, one 2x
            # tensor_tensor max. Stores go out fp16 on the sync (HWDGE)
            # queue, parallel to the SWDGE input queue.
            for ja, jb in poolsegs:
                for b, X, Y, P, R in tiles:
                    y3 = Y[:, :].rearrange("p (q d) -> p q d", d=D)
                    p3 = P[:, :].rearrange("p (j d) -> p j d", d=D)
                    ps = p3[:, ja:jb, :]
                    pf = P[:, ja * D : jb * D]
                    rs = R[:, ja * D : jb * D]
                    nc.vector.tensor_tensor(
                        ps,
                        y3[:, 2 * ja : 2 * jb - 1 : 2, :],
                        y3[:, 2 * ja + 1 : 2 * jb : 2, :],
                        op=ALU.max,
                    )
                    nc.vector.tensor_tensor(
                        ps, ps, y3[:, 2 * ja + 2 : 2 * jb + 1 : 2, :], op=ALU.max
                    )
                    nc.vector.tensor_scalar(
                        rs, pf, -1.0, 0.0, op0=ALU.add, op1=ALU.min
                    )
                    nc.scalar.activation(rs, rs, AF.Exp)
                    nc.vector.tensor_tensor(rs, rs, pf, op=ALU.max)
                    nc.sync.dma_start(
                        out=bass.AP(
                            y,
                            b * ybat + ja * xrow,
                            [[JT * xrow, 128], [1, (jb - ja) * xrow]],
                        ),
                        in_=rs,
                    )
    return nc


def kernel(x: np.ndarray, w: np.ndarray, b: np.ndarray) -> np.ndarray:
    global LAST_RESULT
    w = np.asarray(w, dtype=np.float32)
    bb = np.asarray(b, dtype=np.float32)
    key = (float(w[0]), float(w[1]), float(w[2]), float(bb[0]))
    if key not in _cache:
        _cache[key] = _build(*key)
    nc = _cache[key]

    x = np.asarray(x, dtype=np.float32)
    assert x.shape == (B, L, D), x.shape
    wdiag = np.concatenate(
        [np.eye(128, dtype=np.float16) * np.float16(w[k]) for k in range(3)]
        + [
            np.eye(128, k=1, dtype=np.float16) * np.float16(w[k])
            for k in range(3)
        ],
        axis=1,
    )
    in_maps = [
        {
            "x": np.ascontiguousarray(x[c * BPC : (c + 1) * BPC]),
            "wd": wdiag,
        }
        for c in range(N_CORES)
    ]
    res = run_bass_kernel_spmd(nc, in_maps, core_ids=list(range(N_CORES)))
    LAST_RESULT = res
    out = np.concatenate([r["y"] for r in res.results], axis=0)
    # device computes out+1 in fp16 (see _build); undo the shift here
    return out.astype(np.float32) - 1.0
